# revision 19
# baseline (speedup 1.0000x reference)
"""Multi-head attention on 8 trn2 NeuronCores, head-parallel (2 heads/core).

Math per head h (reference semantics):
  Q = query @ Wq[h] + bq[h];  K = key @ Wk[h] + bk[h];  V = query @ Wv[h] + bv[h]
  P = exp(Q K^T / sqrt(D));  alpha = P / rowsum(P)
  ctx = alpha @ V;  y_h = (ctx @ Wp[h] + bp[h]) @ Wo[h]
  out = sum_h y_h + bo

Device-side formulation:
  Rows of alpha sum to 1, so all linear tails fold into the V projection:
    out = sum_h alpha_h @ (X Wv_h Wp_h Wo_h) + const_bias_row
  Projections and the PV contraction run in fp8-e4m3 with DoubleRow perf
  mode (2 k-tiles per pass = 2x PE throughput); QK^T stays bf16 (its
  contraction is a single 128 k-tile).  Weights are host-scaled into the
  fp8 normal range (wq,wk x32 -> exp scale /1024; wv''=Wv Wp Wo x64 ->
  host output /64).  Unnormalized softmax; rowsum via ones-matmul
  collapse of a DVE-accumulated exp sum; y = sum_h ctx_h / rowsum_h;
  ReduceScatter per 1024-query block; host adds the bias row.

Scheduling: the attention loop is ACT(exp)-bound while projections are
pure-PE, so projection work for batch b+1 is interleaved into the
attention kt-loops of batch b as "units".  DMA issue cost (~600ns per
dma_start on the issuing sequencer) means few, large dma ops.  Queues:
input chunks on sync, y staging on vector, collectives + out dma on
gpsimd (so a waiting ReduceScatter never blocks input DMAs).
"""

import sys

if "/opt/trn_rl_repo" not in sys.path:
    sys.path.insert(0, "/opt/trn_rl_repo")

from collections import deque

import ml_dtypes
import numpy as np

import concourse.mybir as mybir
import concourse.tile as tile
from concourse import bacc
from concourse.bass_utils import run_bass_kernel_spmd

B, S = 4, 2048
IN, D, H = 1024, 128, 16
NCORES = 8
HPC = H // NCORES  # heads per core
NCH = IN // 128  # input chunks
TB = 512  # projection token block
NTB = S // TB
QB = 512  # attention query block
NQB = S // QB
NKT = S // 128  # attention key tiles
NKP = NKT // 2  # key-tile pairs (DoubleRow)
ESH = D // NCORES  # output shard rows per core
NQP = NQB // 2

f32 = mybir.dt.float32
bf16 = mybir.dt.bfloat16
f8 = mybir.dt.float8e4
AF = mybir.ActivationFunctionType
DR = mybir.MatmulPerfMode.DoubleRow

_cache = {}


def build():
    nc = bacc.Bacc(None, target_bir_lowering=False, num_devices=NCORES)

    qT = nc.dram_tensor("qT", [B, IN, S], f8, kind="ExternalInput")
    kT = nc.dram_tensor("kT", [B, IN, S], f8, kind="ExternalInput")
    # prepacked, host-scaled fp8 weights (see kernel())
    wq = nc.dram_tensor("wq", [128, HPC, NCH, D], f8, kind="ExternalInput")
    wk = nc.dram_tensor("wk", [128, HPC, NCH, D], f8, kind="ExternalInput")
    wv = nc.dram_tensor("wv", [128, NCH, HPC, D], f8, kind="ExternalInput")
    bqT = nc.dram_tensor("bqT", [D, HPC], f32, kind="ExternalInput")
    bkT = nc.dram_tensor("bkT", [D, HPC], f32, kind="ExternalInput")
    onemb = nc.dram_tensor("onemb", [D, D], bf16, kind="ExternalInput")

    out_y = nc.dram_tensor("out_y", [B, NQP, ESH, 2 * QB], f32, kind="ExternalOutput")
    y_bounce = [
        [nc.dram_tensor(f"y_bounce{b}_{q}", [D, 2 * QB], f32) for q in range(NQP)]
        for b in range(B)
    ]
    y_shard = [
        [nc.dram_tensor(f"y_shard{b}_{q}", [ESH, 2 * QB], f32) for q in range(NQP)]
        for b in range(B)
    ]

    # Q,K are x32-scaled -> scores x1024
    scale = 1.0 / float(np.sqrt(D)) / 1024.0

    with tile.TileContext(nc) as tc:
        with (
            tc.tile_pool(name="const", bufs=1) as cpool,
            tc.tile_pool(name="xch", bufs=3) as xch,
            tc.tile_pool(name="qkv", bufs=2) as qkv,
            tc.tile_pool(name="work", bufs=2) as work,
            tc.tile_pool(name="pexpp", bufs=6) as pexpp,
            tc.tile_pool(name="psS", bufs=2, space="PSUM") as psS,
            tc.tile_pool(name="psC", bufs=1, space="PSUM") as psC,
            tc.tile_pool(name="psP", bufs=2, space="PSUM") as psP,
        ):
            # ---- resident constants (one DMA op each) ----
            wq_sb = cpool.tile([128, HPC, NCH, D], f8, tag="wq_sb")
            wk_sb = cpool.tile([128, HPC, NCH, D], f8, tag="wk_sb")
            wv_sb = cpool.tile([128, NCH, HPC, D], f8, tag="wv_sb")
            bq_sb = cpool.tile([128, HPC], f32, tag="bq_sb")
            bk_sb = cpool.tile([128, HPC], f32, tag="bk_sb")
            # wk/bk first: the first projection unit is K(h0, tb0)
            nc.sync.dma_start(wk_sb[:], wk[:])
            nc.sync.dma_start(bk_sb[:], bkT[:])
            nc.sync.dma_start(wq_sb[:], wq[:])
            nc.sync.dma_start(bq_sb[:], bqT[:])
            nc.sync.dma_start(wv_sb[:], wv[:])
            onemb_sb = cpool.tile([D, D], bf16, tag="onemb_sb")
            nc.sync.dma_start(onemb_sb[:], onemb[:])

            QTd, KTd, Vnd = {}, {}, {}

            # ---------- projection units ----------
            # Each unit: (batch, cls, dma_thunk, mm_thunk).  cls 'A' must land
            # before attn(batch) starts; cls 'B' (head-1 Q/K) before the first
            # h==1 attention section of the batch.
            def make_units(b):
                QT = QTd[b] = [qkv.tile([128, S], bf16, tag=f"QT{h}", name=f"QT{b}_{h}") for h in range(HPC)]
                KTs = KTd[b] = [qkv.tile([128, S], bf16, tag=f"KT{h}", name=f"KT{b}_{h}") for h in range(HPC)]
                Vn = Vnd[b] = [
                    qkv.tile([128, NKP, 2, 128], f8, tag=f"VN{h}", name=f"VN{b}_{h}")
                    for h in range(HPC)
                ]
                units = []

                def chunk_dma(src, tb):
                    chs = xch.tile([128, NCH, TB], f8, tag="xch", bufs=3)
                    sl = slice(tb * TB, (tb + 1) * TB)
                    nc.sync.dma_start(
                        chs[:], src[b, :, sl].rearrange("(c p) n -> p c n", p=128)
                    )
                    return chs

                def qk_piece(tb, h, w_sb, bias_sb, dst, box):
                    # full 512-token projection block (~1.05us PE; NB 256-token
                    # halves are LDWEIGHTS-bound: DR stationary load is 213ns)
                    sl = slice(tb * TB, (tb + 1) * TB)
                    chs = box[0]
                    pq = psP.tile([128, TB], f32, tag="pP", name="pqk", bufs=2)
                    for cp in range(NCH // 2):
                        nc.tensor.matmul(
                            pq[:],
                            w_sb[:, h, 2 * cp : 2 * cp + 2, :],
                            chs[:, 2 * cp : 2 * cp + 2, :],
                            start=(cp == 0), stop=(cp == NCH // 2 - 1),
                            perf_mode=DR,
                        )
                    with nc.allow_low_precision(reason="f32 psum -> bf16"):
                        nc.vector.tensor_scalar_add(
                            dst[h][:, sl], pq[:], bias_sb[:, h : h + 1]
                        )

                def v_piece(tb, t, box):
                    chs = box[0]
                    pvt = psP.tile([128, 2 * D], f32, tag="pP", name="pvt", bufs=2)
                    for cp in range(NCH // 2):
                        nc.tensor.matmul(
                            pvt[:],
                            chs[:, 2 * cp : 2 * cp + 2, t * 128 : (t + 1) * 128],
                            wv_sb[:, 2 * cp : 2 * cp + 2, :, :],
                            start=(cp == 0), stop=(cp == NCH // 2 - 1),
                            perf_mode=DR,
                        )
                    kt = tb * (TB // 128) + t
                    for h in range(HPC):
                        with nc.allow_low_precision(reason="fp8 PV operand"):
                            nc.vector.tensor_copy(
                                Vn[h][:, kt // 2, kt % 2, :],
                                pvt[:, h * D : (h + 1) * D],
                            )

                def mk(cls, src, tb, pieces):
                    # pieces share one chunk tile; dma on the first piece only
                    box = [None]

                    def dma_thunk(box=box, src=src, tb=tb):
                        box[0] = chunk_dma(src, tb)

                    for i, fn in enumerate(pieces):
                        units.append((b, cls, dma_thunk if i == 0 else None,
                                      (lambda fn=fn, box=box: fn(box))))

                def kp(tb, h):
                    return lambda box: qk_piece(tb, h, wk_sb, bk_sb, KTs, box)

                def qp(tb, h):
                    return lambda box: qk_piece(tb, h, wq_sb, bq_sb, QT, box)

                def vp(tb, t):
                    return lambda box: v_piece(tb, t, box)

                for tb in range(NTB):
                    mk("A", kT, tb, [kp(tb, 0)])
                for tb in range(NTB):
                    mk("A", qT, tb, [qp(tb, 0)] + [vp(tb, t) for t in range(4)])
                for tb in range(NTB):
                    mk("B", kT, tb, [kp(tb, 1)])
                for tb in range(NTB):
                    mk("B", qT, tb, [qp(tb, 1)])
                return units

            # ---------- unit scheduler (dma issued ~2 chunks ahead) ----------
            queue = deque()
            dma_lead = deque()
            LEAD = 6  # pieces; ~2-3 chunk DMAs in flight (xch bufs=3)

            def _top_up():
                while queue and len(dma_lead) < LEAD:
                    u = queue.popleft()
                    if u[2] is not None:
                        u[2]()
                    dma_lead.append(u)

            def pull_one():
                _top_up()
                if dma_lead:
                    u = dma_lead.popleft()
                    u[3]()
                    _top_up()

            def flush(pred):
                _top_up()
                while dma_lead and pred(dma_lead[0]):
                    u = dma_lead.popleft()
                    u[3]()
                    _top_up()

            def push_units(b):
                for u in make_units(b):
                    queue.append(u)
                _top_up()

            # ---------- attention ----------
            def attn_batch(b):
                QT, KTs, Vn = QTd.pop(b), KTd.pop(b), Vnd.pop(b)
                for qbp in range(NQP):
                    q0 = qbp * 2 * QB
                    sl0 = slice(q0, q0 + QB)
                    sl1 = slice(q0 + QB, q0 + 2 * QB)
                    ytile = work.tile([128, 2 * QB], f32, tag="ytile", name="ytile")
                    for h in range(HPC):
                        if h == 1:
                            flush(lambda u: u[0] < b or (u[0] == b and u[1] == "B"))
                        pctx = psC.tile([128, 2 * QB], f32, tag="pCtx", name="pctx", bufs=1)
                        # rowsum accumulators: pair-wide adds split across the
                        # vector and gpsimd engines (DVE alone can't keep up
                        # with fp8-rate adds)
                        acc_v = work.tile([128, 2, 2 * QB], bf16, tag="acc_v", name="acc_v")
                        acc_g = work.tile([128, 2, 2 * QB], bf16, tag="acc_g", name="acc_g")
                        ptiles = []

                        def emit_pv(pair):
                            # PV software-pipelined one pair behind the QK/exp
                            # chain so it overlaps the next pair's exps instead
                            # of serializing inside the dependency loop
                            for half in range(2):
                                hs = slice(half * QB, (half + 1) * QB)
                                nc.tensor.matmul(
                                    pctx[:, hs], Vn[h][:, pair], ptiles[pair][:, :, hs],
                                    start=(pair == 0), stop=(pair == NKP - 1),
                                    perf_mode=DR,
                                )

                        for pair in range(NKP):
                            pexp2 = pexpp.tile([128, 2, 2 * QB], f8, tag="pexp", bufs=8)
                            ptiles.append(pexp2)
                            for sub in range(2):
                                kt = 2 * pair + sub
                                ps2 = psS.tile([128, 2 * QB], f32, tag="pS", name="ps2", bufs=2)
                                ksl = slice(kt * 128, (kt + 1) * 128)
                                nc.tensor.matmul(
                                    ps2[:, :QB], KTs[h][:, ksl], QT[h][:, sl0],
                                    start=True, stop=True,
                                )
                                nc.tensor.matmul(
                                    ps2[:, QB:], KTs[h][:, ksl], QT[h][:, sl1],
                                    start=True, stop=True,
                                )
                                nc.scalar.activation(
                                    pexp2[:, sub, :], ps2[:], AF.Exp, scale=scale
                                )
                            if pair > 0:
                                emit_pv(pair - 1)
                            pull_one()  # proj filler where PE waits on exp
                            with nc.allow_low_precision(reason="bf16 rowsum acc"):
                                if pair == 2:
                                    nc.vector.tensor_add(acc_v[:], ptiles[0][:], ptiles[2][:])
                                elif pair == 3:
                                    nc.gpsimd.tensor_add(acc_g[:], ptiles[1][:], ptiles[3][:])
                                elif pair in (4, 6):
                                    nc.vector.tensor_add(acc_v[:], acc_v[:], ptiles[pair][:])
                                elif pair in (5, 7):
                                    nc.gpsimd.tensor_add(acc_g[:], acc_g[:], ptiles[pair][:])
                        emit_pv(NKP - 1)
                        # rowsum collapse + normalize, pipelined per 512-half
                        rsbr = work.tile([128, 2 * QB], f32, tag="rsbr", name="rsbr", bufs=2)
                        ctxn = None if h == 0 else work.tile(
                            [128, 2 * QB], f32, tag="ctxn", name="ctxn"
                        )
                        for half in range(2):
                            hs = slice(half * QB, (half + 1) * QB)
                            pbc = psP.tile([128, QB], f32, tag="pP", name="pbc", bufs=2)
                            srcs = [acc_v[:, 0, hs], acc_v[:, 1, hs], acc_g[:, 0, hs], acc_g[:, 1, hs]]
                            for si, src in enumerate(srcs):
                                nc.tensor.matmul(
                                    pbc[:], onemb_sb[:], src,
                                    start=(si == 0), stop=(si == len(srcs) - 1),
                                )
                            nc.vector.reciprocal_approx_fast(out=rsbr[:, hs], in_=pbc[:])
                            if h == 0:
                                nc.vector.tensor_mul(ytile[:, hs], pctx[:, hs], rsbr[:, hs])
                            else:
                                nc.vector.tensor_mul(ctxn[:, hs], pctx[:, hs], rsbr[:, hs])
                                nc.vector.tensor_add(
                                    ytile[:, hs], ytile[:, hs], ctxn[:, hs]
                                )
                                nc.sync.dma_start(
                                    y_bounce[b][qbp][:, hs], ytile[:, hs]
                                )
                            if half == 0:
                                pull_one()
                        if h == 1:
                            nc.gpsimd.collective_compute(
                                "ReduceScatter",
                                mybir.AluOpType.add,
                                replica_groups=[list(range(NCORES))],
                                ins=[y_bounce[b][qbp][:].opt()],
                                outs=[y_shard[b][qbp][:].opt()],
                            )

            # ---------- schedule ----------
            push_units(0)
            flush(lambda u: u[0] == 0 and u[1] == "A")
            for b in range(B):
                if b + 1 < B:
                    push_units(b + 1)
                attn_batch(b)
                flush(lambda u: u[0] <= b)
            flush(lambda u: True)
            # out dmas at the end: each waits its RS; nothing queues behind
            for b in range(B):
                for qbp in range(NQP):
                    nc.sync.dma_start(out_y[b, qbp], y_shard[b][qbp][:])

    nc.compile()
    return nc


def kernel(**inputs):
    query = np.asarray(inputs["query"], np.float32)
    key = np.asarray(inputs["key"], np.float32)
    Wq, bq = np.asarray(inputs["Wq"], np.float32), np.asarray(inputs["bq"], np.float32)
    Wk, bk = np.asarray(inputs["Wk"], np.float32), np.asarray(inputs["bk"], np.float32)
    Wv, bv = np.asarray(inputs["Wv"], np.float32), np.asarray(inputs["bv"], np.float32)
    Wp, bp = np.asarray(inputs["Wp"], np.float32), np.asarray(inputs["bp"], np.float32)
    Wo, bo = np.asarray(inputs["Wo"], np.float32), np.asarray(inputs["bo"], np.float32)

    f8np = ml_dtypes.float8_e4m3
    qT_8 = np.ascontiguousarray(query.transpose(0, 2, 1)).astype(f8np)
    kT_8 = np.ascontiguousarray(key.transpose(0, 2, 1)).astype(f8np)

    if "nc" not in _cache:
        _cache["nc"] = build()
    nc = _cache["nc"]

    def prepack_qk(w):  # 32x-scaled [HPC, IN, D] -> [128, HPC, NCH, D] fp8
        return np.ascontiguousarray(
            (32.0 * w).reshape(HPC, NCH, 128, D).transpose(2, 0, 1, 3)
        ).astype(f8np)

    def prepack_v(w):  # 64x-scaled [HPC, IN, D] -> [128, NCH, HPC, D] fp8
        return np.ascontiguousarray(
            (64.0 * w).reshape(HPC, NCH, 128, D).transpose(2, 1, 0, 3)
        ).astype(f8np)

    Wo_h = Wo.reshape(H, D, D)
    bias_total = (
        np.einsum("hd,hde,hef->f", bv.astype(np.float64), Wp.astype(np.float64), Wo_h.astype(np.float64))
        + np.einsum("hd,hdf->f", bp.astype(np.float64), Wo_h.astype(np.float64))
        + bo.astype(np.float64)
    ).astype(np.float32)

    in_maps = []
    for i in range(NCORES):
        hs = slice(i * HPC, (i + 1) * HPC)
        wvpp = np.einsum(
            "hid,hde,hef->hif",
            Wv[hs].astype(np.float64),
            Wp[hs].astype(np.float64),
            Wo_h[hs].astype(np.float64),
        ).astype(np.float32)
        in_maps.append(
            {
                "qT": qT_8,
                "kT": kT_8,
                "wq": prepack_qk(Wq[hs]),
                "wk": prepack_qk(Wk[hs]),
                "wv": prepack_v(wvpp),
                "bqT": np.ascontiguousarray(32.0 * bq[hs].T),
                "bkT": np.ascontiguousarray(32.0 * bk[hs].T),
                "onemb": np.ones((D, D), ml_dtypes.bfloat16),
            }
        )

    res = run_bass_kernel_spmd(nc, in_maps, core_ids=list(range(NCORES)))
    _cache["last_result"] = res
    # shards: per core [B, NQP, ESH, 2QB] -> full [B, S, D]; y is 64x-scaled
    parts = np.stack([res.results[i]["out_y"] for i in range(NCORES)], axis=2)
    yfull = parts.reshape(B, NQP, D, 2 * QB).transpose(0, 1, 3, 2).reshape(B, S, D)
    return np.ascontiguousarray(yfull / 64.0 + bias_total[None, None, :])


# revision 22
# speedup vs baseline: 1.0890x; 1.0890x over previous
"""Multi-head attention on 8 trn2 NeuronCores, head-parallel (2 heads/core).

Math per head h (reference semantics):
  Q = query @ Wq[h] + bq[h];  K = key @ Wk[h] + bk[h];  V = query @ Wv[h] + bv[h]
  P = exp(Q K^T / sqrt(D));  alpha = P / rowsum(P)
  ctx = alpha @ V;  y_h = (ctx @ Wp[h] + bp[h]) @ Wo[h]
  out = sum_h y_h + bo

Device-side formulation:
  Rows of alpha sum to 1, so all linear tails fold into the V projection:
    out = sum_h alpha_h @ (X Wv_h Wp_h Wo_h) + const_bias_row
  Projections and the PV contraction run in fp8-e4m3 with DoubleRow perf
  mode (2 k-tiles per pass = 2x PE throughput); QK^T stays bf16 (its
  contraction is a single 128 k-tile).  Weights are host-scaled into the
  fp8 normal range (wq,wk x32 -> exp scale /1024; wv''=Wv Wp Wo x64 ->
  host output /64).  Unnormalized softmax; rowsum via ones-matmul
  collapse of a DVE-accumulated exp sum; y = sum_h ctx_h / rowsum_h;
  ReduceScatter per 1024-query block; host adds the bias row.

Scheduling: the attention loop is ACT(exp)-bound while projections are
pure-PE, so projection work for batch b+1 is interleaved into the
attention kt-loops of batch b as "units".  DMA issue cost (~600ns per
dma_start on the issuing sequencer) means few, large dma ops.  Queues:
input chunks on sync, y staging on vector, collectives + out dma on
gpsimd (so a waiting ReduceScatter never blocks input DMAs).
"""

import sys

if "/opt/trn_rl_repo" not in sys.path:
    sys.path.insert(0, "/opt/trn_rl_repo")

from collections import deque

import ml_dtypes
import numpy as np

import concourse.mybir as mybir
import concourse.tile as tile
from concourse import bacc
from concourse.bass_utils import run_bass_kernel_spmd

B, S = 4, 2048
IN, D, H = 1024, 128, 16
NCORES = 8
HPC = H // NCORES  # heads per core
NCH = IN // 128  # input chunks
TB = 512  # projection token block
NTB = S // TB
QB = 512  # attention query block
NQB = S // QB
NKT = S // 128  # attention key tiles
NKP = NKT // 2  # key-tile pairs (DoubleRow)
ESH = D // NCORES  # output shard rows per core
NQP = NQB // 2

f32 = mybir.dt.float32
bf16 = mybir.dt.bfloat16
f8 = mybir.dt.float8e4
AF = mybir.ActivationFunctionType
DR = mybir.MatmulPerfMode.DoubleRow

_cache = {}


def build():
    nc = bacc.Bacc(None, target_bir_lowering=False, num_devices=NCORES)

    qT = nc.dram_tensor("qT", [B, IN, S], f8, kind="ExternalInput")
    kT = nc.dram_tensor("kT", [B, IN, S], f8, kind="ExternalInput")
    # prepacked, host-scaled fp8 weights (see kernel())
    wq = nc.dram_tensor("wq", [128, HPC, NCH, D], f8, kind="ExternalInput")
    wk = nc.dram_tensor("wk", [128, HPC, NCH, D], f8, kind="ExternalInput")
    wv = nc.dram_tensor("wv", [128, NCH, HPC, D], f8, kind="ExternalInput")
    bqT = nc.dram_tensor("bqT", [D, HPC], f32, kind="ExternalInput")
    bkT = nc.dram_tensor("bkT", [D, HPC], f32, kind="ExternalInput")
    onemb = nc.dram_tensor("onemb", [D, D], bf16, kind="ExternalInput")

    out_y = nc.dram_tensor("out_y", [B, NQP, ESH, 2 * QB], f32, kind="ExternalOutput")
    y_bounce = [
        [nc.dram_tensor(f"y_bounce{b}_{q}", [D, 2 * QB], f32) for q in range(NQP)]
        for b in range(B)
    ]
    y_shard = [
        [nc.dram_tensor(f"y_shard{b}_{q}", [ESH, 2 * QB], f32) for q in range(NQP)]
        for b in range(B)
    ]

    # Q,K are x32-scaled -> scores x1024
    scale = 1.0 / float(np.sqrt(D)) / 1024.0

    with tile.TileContext(nc) as tc:
        with (
            tc.tile_pool(name="const", bufs=1) as cpool,
            tc.tile_pool(name="xch", bufs=3) as xch,
            tc.tile_pool(name="qkv", bufs=2) as qkv,
            tc.tile_pool(name="work", bufs=2) as work,
            tc.tile_pool(name="pexpp", bufs=6) as pexpp,
            tc.tile_pool(name="psS", bufs=2, space="PSUM") as psS,
            tc.tile_pool(name="psC", bufs=1, space="PSUM") as psC,
            tc.tile_pool(name="psP", bufs=2, space="PSUM") as psP,
        ):
            # ---- resident constants (one DMA op each) ----
            wq_sb = cpool.tile([128, HPC, NCH, D], f8, tag="wq_sb")
            wk_sb = cpool.tile([128, HPC, NCH, D], f8, tag="wk_sb")
            wv_sb = cpool.tile([128, NCH, HPC, D], f8, tag="wv_sb")
            bq_sb = cpool.tile([128, HPC], f32, tag="bq_sb")
            bk_sb = cpool.tile([128, HPC], f32, tag="bk_sb")
            # wk/bk first: the first projection unit is K(h0, tb0)
            nc.sync.dma_start(wk_sb[:], wk[:])
            nc.sync.dma_start(bk_sb[:], bkT[:])
            nc.sync.dma_start(wq_sb[:], wq[:])
            nc.sync.dma_start(bq_sb[:], bqT[:])
            nc.sync.dma_start(wv_sb[:], wv[:])
            onemb_sb = cpool.tile([D, D], bf16, tag="onemb_sb")
            nc.sync.dma_start(onemb_sb[:], onemb[:])

            QTd, KTd, Vnd = {}, {}, {}

            # ---------- projection units ----------
            # Each unit: (batch, cls, dma_thunk, mm_thunk).  cls 'A' must land
            # before attn(batch) starts; cls 'B' (head-1 Q/K) before the first
            # h==1 attention section of the batch.
            def make_units(b):
                QT = QTd[b] = [qkv.tile([128, S], bf16, tag=f"QT{h}", name=f"QT{b}_{h}") for h in range(HPC)]
                KTs = KTd[b] = [qkv.tile([128, S], bf16, tag=f"KT{h}", name=f"KT{b}_{h}") for h in range(HPC)]
                Vn = Vnd[b] = [
                    qkv.tile([128, NKP, 2, 128], f8, tag=f"VN{h}", name=f"VN{b}_{h}")
                    for h in range(HPC)
                ]
                units = []

                def chunk_dma(src, tb):
                    chs = xch.tile([128, NCH, TB], f8, tag="xch", bufs=3)
                    sl = slice(tb * TB, (tb + 1) * TB)
                    nc.sync.dma_start(
                        chs[:], src[b, :, sl].rearrange("(c p) n -> p c n", p=128)
                    )
                    return chs

                def qk_piece(tb, h, w_sb, bias_sb, dst, box):
                    # full 512-token projection block (~1.05us PE; NB 256-token
                    # halves are LDWEIGHTS-bound: DR stationary load is 213ns)
                    sl = slice(tb * TB, (tb + 1) * TB)
                    chs = box[0]
                    pq = psP.tile([128, TB], f32, tag="pP", name="pqk", bufs=2)
                    for cp in range(NCH // 2):
                        nc.tensor.matmul(
                            pq[:],
                            w_sb[:, h, 2 * cp : 2 * cp + 2, :],
                            chs[:, 2 * cp : 2 * cp + 2, :],
                            start=(cp == 0), stop=(cp == NCH // 2 - 1),
                            perf_mode=DR,
                        )
                    with nc.allow_low_precision(reason="f32 psum -> bf16"):
                        nc.vector.tensor_scalar_add(
                            dst[h][:, sl], pq[:], bias_sb[:, h : h + 1]
                        )

                def v_piece(tb, t, box):
                    chs = box[0]
                    pvt = psP.tile([128, 2 * D], f32, tag="pP", name="pvt", bufs=2)
                    for cp in range(NCH // 2):
                        nc.tensor.matmul(
                            pvt[:],
                            chs[:, 2 * cp : 2 * cp + 2, t * 128 : (t + 1) * 128],
                            wv_sb[:, 2 * cp : 2 * cp + 2, :, :],
                            start=(cp == 0), stop=(cp == NCH // 2 - 1),
                            perf_mode=DR,
                        )
                    kt = tb * (TB // 128) + t
                    for h in range(HPC):
                        with nc.allow_low_precision(reason="fp8 PV operand"):
                            nc.vector.tensor_copy(
                                Vn[h][:, kt // 2, kt % 2, :],
                                pvt[:, h * D : (h + 1) * D],
                            )

                def mk(cls, src, tb, pieces):
                    # pieces share one chunk tile; dma on the first piece only
                    box = [None]

                    def dma_thunk(box=box, src=src, tb=tb):
                        box[0] = chunk_dma(src, tb)

                    for i, fn in enumerate(pieces):
                        units.append((b, cls, dma_thunk if i == 0 else None,
                                      (lambda fn=fn, box=box: fn(box))))

                def kp(tb, h):
                    return lambda box: qk_piece(tb, h, wk_sb, bk_sb, KTs, box)

                def qp(tb, h):
                    return lambda box: qk_piece(tb, h, wq_sb, bq_sb, QT, box)

                def vp(tb, t):
                    return lambda box: v_piece(tb, t, box)

                for tb in range(NTB):
                    mk("A", kT, tb, [kp(tb, 0)])
                for tb in range(NTB):
                    mk("A", qT, tb, [qp(tb, 0)] + [vp(tb, t) for t in range(4)])
                for tb in range(NTB):
                    mk("B", kT, tb, [kp(tb, 1)])
                for tb in range(NTB):
                    mk("B", qT, tb, [qp(tb, 1)])
                return units

            # ---------- unit scheduler (dma issued ~2 chunks ahead) ----------
            queue = deque()
            dma_lead = deque()
            LEAD = 6  # pieces; ~2-3 chunk DMAs in flight (xch bufs=3)

            def _top_up():
                while queue and len(dma_lead) < LEAD:
                    u = queue.popleft()
                    if u[2] is not None:
                        u[2]()
                    dma_lead.append(u)

            def pull_one():
                _top_up()
                if dma_lead:
                    u = dma_lead.popleft()
                    u[3]()
                    _top_up()

            def flush(pred):
                _top_up()
                while dma_lead and pred(dma_lead[0]):
                    u = dma_lead.popleft()
                    u[3]()
                    _top_up()

            def push_units(b):
                for u in make_units(b):
                    queue.append(u)
                _top_up()

            # ---------- attention ----------
            def attn_batch(b):
                QT, KTs, Vn = QTd.pop(b), KTd.pop(b), Vnd.pop(b)
                for qbp in range(NQP):
                    q0 = qbp * 2 * QB
                    sl0 = slice(q0, q0 + QB)
                    sl1 = slice(q0 + QB, q0 + 2 * QB)
                    ytile = work.tile([128, 2 * QB], f32, tag="ytile", name="ytile")
                    for h in range(HPC):
                        if h == 1:
                            flush(lambda u: u[0] < b or (u[0] == b and u[1] == "B"))
                        pctx = psC.tile([128, 2 * QB], f32, tag="pCtx", name="pctx", bufs=1)
                        # rowsum accumulators: pair-wide adds split across the
                        # vector and gpsimd engines (DVE alone can't keep up
                        # with fp8-rate adds)
                        # flat 2D accumulators: 3D-strided adds run ~3x slower
                        acc_v = work.tile([128, 4 * QB], bf16, tag="acc_v", name="acc_v")
                        acc_g = work.tile([128, 4 * QB], bf16, tag="acc_g", name="acc_g")
                        ptiles = []

                        def emit_pv(pair):
                            # PV software-pipelined one pair behind the QK/exp
                            # chain so it overlaps the next pair's exps instead
                            # of serializing inside the dependency loop
                            for half in range(2):
                                hs = slice(half * QB, (half + 1) * QB)
                                nc.tensor.matmul(
                                    pctx[:, hs], Vn[h][:, pair], ptiles[pair][:, :, hs],
                                    start=(pair == 0), stop=(pair == NKP - 1),
                                    perf_mode=DR,
                                )

                        for pair in range(NKP):
                            pexp2 = pexpp.tile([128, 2, 2 * QB], f8, tag="pexp", bufs=8)
                            ptiles.append(pexp2)
                            for sub in range(2):
                                kt = 2 * pair + sub
                                ps2 = psS.tile([128, 2 * QB], f32, tag="pS", name="ps2", bufs=2)
                                ksl = slice(kt * 128, (kt + 1) * 128)
                                nc.tensor.matmul(
                                    ps2[:, :QB], KTs[h][:, ksl], QT[h][:, sl0],
                                    start=True, stop=True,
                                )
                                nc.tensor.matmul(
                                    ps2[:, QB:], KTs[h][:, ksl], QT[h][:, sl1],
                                    start=True, stop=True,
                                )
                                nc.scalar.activation(
                                    pexp2[:, sub, :], ps2[:], AF.Exp, scale=scale
                                )
                            if pair > 0:
                                emit_pv(pair - 1)
                            pull_one()  # proj filler where PE waits on exp

                            def pm(i):
                                return ptiles[i][:].rearrange("p a q -> p (a q)")

                            # last pair's add on the faster DVE so the rowsum
                            # collapse isn't stalled at the head boundary
                            with nc.allow_low_precision(reason="bf16 rowsum acc"):
                                if pair == 2:
                                    nc.vector.tensor_add(acc_v[:], pm(0), pm(2))
                                elif pair == 3:
                                    nc.gpsimd.tensor_add(acc_g[:], pm(1), pm(3))
                                elif pair == 5:
                                    nc.gpsimd.tensor_add(acc_g[:], acc_g[:], pm(5))
                                elif pair in (4, 6, 7):
                                    nc.vector.tensor_add(acc_v[:], acc_v[:], pm(pair))
                        emit_pv(NKP - 1)
                        pull_one()
                        # rowsum collapse + normalize, pipelined per 512-half
                        rsbr = work.tile([128, 2 * QB], f32, tag="rsbr", name="rsbr", bufs=2)
                        ctxn = None if h == 0 else work.tile(
                            [128, 2 * QB], f32, tag="ctxn", name="ctxn"
                        )
                        for half in range(2):
                            hs = slice(half * QB, (half + 1) * QB)
                            pbc = psP.tile([128, QB], f32, tag="pP", name="pbc", bufs=2)
                            srcs = [
                                acc_v[:, half * QB : (half + 1) * QB],
                                acc_v[:, 2 * QB + half * QB : 2 * QB + (half + 1) * QB],
                                acc_g[:, half * QB : (half + 1) * QB],
                                acc_g[:, 2 * QB + half * QB : 2 * QB + (half + 1) * QB],
                            ]
                            for si, src in enumerate(srcs):
                                nc.tensor.matmul(
                                    pbc[:], onemb_sb[:], src,
                                    start=(si == 0), stop=(si == len(srcs) - 1),
                                )
                            nc.vector.reciprocal_approx_fast(out=rsbr[:, hs], in_=pbc[:])
                            if h == 0:
                                nc.vector.tensor_mul(ytile[:, hs], pctx[:, hs], rsbr[:, hs])
                            else:
                                nc.vector.tensor_mul(ctxn[:, hs], pctx[:, hs], rsbr[:, hs])
                                nc.vector.tensor_add(
                                    ytile[:, hs], ytile[:, hs], ctxn[:, hs]
                                )
                                nc.sync.dma_start(
                                    y_bounce[b][qbp][:, hs], ytile[:, hs]
                                )
                            if half == 0:
                                pull_one()
                        if h == 1:
                            nc.gpsimd.collective_compute(
                                "ReduceScatter",
                                mybir.AluOpType.add,
                                replica_groups=[list(range(NCORES))],
                                ins=[y_bounce[b][qbp][:].opt()],
                                outs=[y_shard[b][qbp][:].opt()],
                            )

            # ---------- schedule ----------
            push_units(0)
            flush(lambda u: u[0] == 0 and u[1] == "A")
            for b in range(B):
                if b + 1 < B:
                    push_units(b + 1)
                attn_batch(b)
                flush(lambda u: u[0] <= b)
            flush(lambda u: True)
            # out dmas at the end: each waits its RS; nothing queues behind
            for b in range(B):
                for qbp in range(NQP):
                    nc.sync.dma_start(out_y[b, qbp], y_shard[b][qbp][:])

    nc.compile()
    return nc


def kernel(**inputs):
    query = np.asarray(inputs["query"], np.float32)
    key = np.asarray(inputs["key"], np.float32)
    Wq, bq = np.asarray(inputs["Wq"], np.float32), np.asarray(inputs["bq"], np.float32)
    Wk, bk = np.asarray(inputs["Wk"], np.float32), np.asarray(inputs["bk"], np.float32)
    Wv, bv = np.asarray(inputs["Wv"], np.float32), np.asarray(inputs["bv"], np.float32)
    Wp, bp = np.asarray(inputs["Wp"], np.float32), np.asarray(inputs["bp"], np.float32)
    Wo, bo = np.asarray(inputs["Wo"], np.float32), np.asarray(inputs["bo"], np.float32)

    f8np = ml_dtypes.float8_e4m3
    qT_8 = np.ascontiguousarray(query.transpose(0, 2, 1)).astype(f8np)
    kT_8 = np.ascontiguousarray(key.transpose(0, 2, 1)).astype(f8np)

    if "nc" not in _cache:
        _cache["nc"] = build()
    nc = _cache["nc"]

    def prepack_qk(w):  # 32x-scaled [HPC, IN, D] -> [128, HPC, NCH, D] fp8
        return np.ascontiguousarray(
            (32.0 * w).reshape(HPC, NCH, 128, D).transpose(2, 0, 1, 3)
        ).astype(f8np)

    def prepack_v(w):  # 64x-scaled [HPC, IN, D] -> [128, NCH, HPC, D] fp8
        return np.ascontiguousarray(
            (64.0 * w).reshape(HPC, NCH, 128, D).transpose(2, 1, 0, 3)
        ).astype(f8np)

    Wo_h = Wo.reshape(H, D, D)
    bias_total = (
        np.einsum("hd,hde,hef->f", bv.astype(np.float64), Wp.astype(np.float64), Wo_h.astype(np.float64))
        + np.einsum("hd,hdf->f", bp.astype(np.float64), Wo_h.astype(np.float64))
        + bo.astype(np.float64)
    ).astype(np.float32)

    in_maps = []
    for i in range(NCORES):
        hs = slice(i * HPC, (i + 1) * HPC)
        wvpp = np.einsum(
            "hid,hde,hef->hif",
            Wv[hs].astype(np.float64),
            Wp[hs].astype(np.float64),
            Wo_h[hs].astype(np.float64),
        ).astype(np.float32)
        in_maps.append(
            {
                "qT": qT_8,
                "kT": kT_8,
                "wq": prepack_qk(Wq[hs]),
                "wk": prepack_qk(Wk[hs]),
                "wv": prepack_v(wvpp),
                "bqT": np.ascontiguousarray(32.0 * bq[hs].T),
                "bkT": np.ascontiguousarray(32.0 * bk[hs].T),
                "onemb": np.ones((D, D), ml_dtypes.bfloat16),
            }
        )

    res = run_bass_kernel_spmd(nc, in_maps, core_ids=list(range(NCORES)))
    _cache["last_result"] = res
    # shards: per core [B, NQP, ESH, 2QB] -> full [B, S, D]; y is 64x-scaled
    parts = np.stack([res.results[i]["out_y"] for i in range(NCORES)], axis=2)
    yfull = parts.reshape(B, NQP, D, 2 * QB).transpose(0, 1, 3, 2).reshape(B, S, D)
    return np.ascontiguousarray(yfull / 64.0 + bias_total[None, None, :])


# revision 24
# speedup vs baseline: 1.1126x; 1.0217x over previous
"""Multi-head attention on 8 trn2 NeuronCores, head-parallel (2 heads/core).

Math per head h (reference semantics):
  Q = query @ Wq[h] + bq[h];  K = key @ Wk[h] + bk[h];  V = query @ Wv[h] + bv[h]
  P = exp(Q K^T / sqrt(D));  alpha = P / rowsum(P)
  ctx = alpha @ V;  y_h = (ctx @ Wp[h] + bp[h]) @ Wo[h]
  out = sum_h y_h + bo

Device-side formulation:
  Rows of alpha sum to 1, so all linear tails fold into the V projection:
    out = sum_h alpha_h @ (X Wv_h Wp_h Wo_h) + const_bias_row
  Projections and the PV contraction run in fp8-e4m3 with DoubleRow perf
  mode (2 k-tiles per pass = 2x PE throughput); QK^T stays bf16 (its
  contraction is a single 128 k-tile).  Weights are host-scaled into the
  fp8 normal range (wq,wk x32 -> exp scale /1024; wv''=Wv Wp Wo x64 ->
  host output /64).  Unnormalized softmax; rowsum via ones-matmul
  collapse of a DVE-accumulated exp sum; y = sum_h ctx_h / rowsum_h;
  ReduceScatter per 1024-query block; host adds the bias row.

Scheduling: the attention loop is ACT(exp)-bound while projections are
pure-PE, so projection work for batch b+1 is interleaved into the
attention kt-loops of batch b as "units".  DMA issue cost (~600ns per
dma_start on the issuing sequencer) means few, large dma ops.  Queues:
input chunks on sync, y staging on vector, collectives + out dma on
gpsimd (so a waiting ReduceScatter never blocks input DMAs).
"""

import sys

if "/opt/trn_rl_repo" not in sys.path:
    sys.path.insert(0, "/opt/trn_rl_repo")

from collections import deque

import ml_dtypes
import numpy as np

import concourse.mybir as mybir
import concourse.tile as tile
from concourse import bacc
from concourse.bass_utils import run_bass_kernel_spmd

B, S = 4, 2048
IN, D, H = 1024, 128, 16
NCORES = 8
HPC = H // NCORES  # heads per core
NCH = IN // 128  # input chunks
TB = 512  # projection token block
NTB = S // TB
QB = 512  # attention query block
NQB = S // QB
NKT = S // 128  # attention key tiles
NKP = NKT // 2  # key-tile pairs (DoubleRow)
ESH = D // NCORES  # output shard rows per core
NQP = NQB // 2

f32 = mybir.dt.float32
bf16 = mybir.dt.bfloat16
f8 = mybir.dt.float8e4
AF = mybir.ActivationFunctionType
DR = mybir.MatmulPerfMode.DoubleRow

_cache = {}


def build():
    nc = bacc.Bacc(None, target_bir_lowering=False, num_devices=NCORES)

    qT = nc.dram_tensor("qT", [B, IN, S], f8, kind="ExternalInput")
    kT = nc.dram_tensor("kT", [B, IN, S], f8, kind="ExternalInput")
    # prepacked, host-scaled fp8 weights (see kernel())
    wq = nc.dram_tensor("wq", [128, HPC, NCH, D], f8, kind="ExternalInput")
    wk = nc.dram_tensor("wk", [128, HPC, NCH, D], f8, kind="ExternalInput")
    wv = nc.dram_tensor("wv", [128, NCH, HPC, D], f8, kind="ExternalInput")
    bqT = nc.dram_tensor("bqT", [D, HPC], f32, kind="ExternalInput")
    bkT = nc.dram_tensor("bkT", [D, HPC], f32, kind="ExternalInput")
    onemb = nc.dram_tensor("onemb", [D, D], bf16, kind="ExternalInput")

    out_y = nc.dram_tensor("out_y", [B, NQP, ESH, 2 * QB], f32, kind="ExternalOutput")
    y_bounce = [
        [nc.dram_tensor(f"y_bounce{b}_{q}", [D, 2 * QB], f32) for q in range(NQP)]
        for b in range(B)
    ]
    y_shard = [
        [nc.dram_tensor(f"y_shard{b}_{q}", [ESH, 2 * QB], f32) for q in range(NQP)]
        for b in range(B)
    ]

    # Q,K are x32-scaled -> scores x1024
    scale = 1.0 / float(np.sqrt(D)) / 1024.0

    with tile.TileContext(nc) as tc:
        with (
            tc.tile_pool(name="const", bufs=1) as cpool,
            tc.tile_pool(name="xch", bufs=3) as xch,
            tc.tile_pool(name="qkv", bufs=2) as qkv,
            tc.tile_pool(name="work", bufs=2) as work,
            tc.tile_pool(name="pexpp", bufs=6) as pexpp,
            tc.tile_pool(name="psS", bufs=2, space="PSUM") as psS,
            tc.tile_pool(name="psC", bufs=1, space="PSUM") as psC,
            tc.tile_pool(name="psP", bufs=2, space="PSUM") as psP,
        ):
            # ---- resident constants (one DMA op each) ----
            wq_sb = cpool.tile([128, HPC, NCH, D], f8, tag="wq_sb")
            wk_sb = cpool.tile([128, HPC, NCH, D], f8, tag="wk_sb")
            wv_sb = cpool.tile([128, NCH, HPC, D], f8, tag="wv_sb")
            bq_sb = cpool.tile([128, HPC], f32, tag="bq_sb")
            bk_sb = cpool.tile([128, HPC], f32, tag="bk_sb")
            # wk/bk first: the first projection unit is K(h0, tb0)
            nc.sync.dma_start(wk_sb[:], wk[:])
            nc.sync.dma_start(bk_sb[:], bkT[:])
            nc.sync.dma_start(wq_sb[:], wq[:])
            nc.sync.dma_start(bq_sb[:], bqT[:])
            nc.sync.dma_start(wv_sb[:], wv[:])
            onemb_sb = cpool.tile([D, D], bf16, tag="onemb_sb")
            nc.sync.dma_start(onemb_sb[:], onemb[:])

            QTd, KTd, Vnd = {}, {}, {}

            # ---------- projection units ----------
            # Each unit: (batch, cls, dma_thunk, mm_thunk).  cls 'A' must land
            # before attn(batch) starts; cls 'B' (head-1 Q/K) before the first
            # h==1 attention section of the batch.
            def make_units(b):
                QT = QTd[b] = [qkv.tile([128, S], bf16, tag=f"QT{h}", name=f"QT{b}_{h}") for h in range(HPC)]
                KTs = KTd[b] = [qkv.tile([128, S], bf16, tag=f"KT{h}", name=f"KT{b}_{h}") for h in range(HPC)]
                Vn = Vnd[b] = [
                    qkv.tile([128, NKP, 2, 128], f8, tag=f"VN{h}", name=f"VN{b}_{h}")
                    for h in range(HPC)
                ]
                units = []

                def chunk_dma(src, tb):
                    chs = xch.tile([128, NCH, TB], f8, tag="xch", bufs=3)
                    sl = slice(tb * TB, (tb + 1) * TB)
                    nc.sync.dma_start(
                        chs[:], src[b, :, sl].rearrange("(c p) n -> p c n", p=128)
                    )
                    return chs

                def qk_piece(tb, h, w_sb, bias_sb, dst, box):
                    # full 512-token projection block (~1.05us PE; NB 256-token
                    # halves are LDWEIGHTS-bound: DR stationary load is 213ns)
                    sl = slice(tb * TB, (tb + 1) * TB)
                    chs = box[0]
                    pq = psP.tile([128, TB], f32, tag="pP", name="pqk", bufs=2)
                    for cp in range(NCH // 2):
                        nc.tensor.matmul(
                            pq[:],
                            w_sb[:, h, 2 * cp : 2 * cp + 2, :],
                            chs[:, 2 * cp : 2 * cp + 2, :],
                            start=(cp == 0), stop=(cp == NCH // 2 - 1),
                            perf_mode=DR,
                        )
                    with nc.allow_low_precision(reason="f32 psum -> bf16"):
                        nc.vector.tensor_scalar_add(
                            dst[h][:, sl], pq[:], bias_sb[:, h : h + 1]
                        )

                def v_piece(tb, t, box):
                    chs = box[0]
                    pvt = psP.tile([128, 2 * D], f32, tag="pP", name="pvt", bufs=2)
                    for cp in range(NCH // 2):
                        nc.tensor.matmul(
                            pvt[:],
                            chs[:, 2 * cp : 2 * cp + 2, t * 128 : (t + 1) * 128],
                            wv_sb[:, 2 * cp : 2 * cp + 2, :, :],
                            start=(cp == 0), stop=(cp == NCH // 2 - 1),
                            perf_mode=DR,
                        )
                    kt = tb * (TB // 128) + t
                    for h in range(HPC):
                        with nc.allow_low_precision(reason="fp8 PV operand"):
                            nc.vector.tensor_copy(
                                Vn[h][:, kt // 2, kt % 2, :],
                                pvt[:, h * D : (h + 1) * D],
                            )

                def mk(cls, src, tb, pieces):
                    # pieces share one chunk tile; dma on the first piece only
                    box = [None]

                    def dma_thunk(box=box, src=src, tb=tb):
                        box[0] = chunk_dma(src, tb)

                    for i, fn in enumerate(pieces):
                        units.append((b, cls, dma_thunk if i == 0 else None,
                                      (lambda fn=fn, box=box: fn(box))))

                def kp(tb, h):
                    return lambda box: qk_piece(tb, h, wk_sb, bk_sb, KTs, box)

                def qp(tb, h):
                    return lambda box: qk_piece(tb, h, wq_sb, bq_sb, QT, box)

                def vp(tb, t):
                    return lambda box: v_piece(tb, t, box)

                for tb in range(NTB):
                    mk("A", kT, tb, [kp(tb, 0)])
                for tb in range(NTB):
                    mk("A", qT, tb, [qp(tb, 0)] + [vp(tb, t) for t in range(4)])
                for tb in range(NTB):
                    mk("B", kT, tb, [kp(tb, 1)])
                for tb in range(NTB):
                    mk("B", qT, tb, [qp(tb, 1)])
                return units

            # ---------- unit scheduler (dma issued ~2 chunks ahead) ----------
            queue = deque()
            dma_lead = deque()
            LEAD = 6  # pieces; ~2-3 chunk DMAs in flight (xch bufs=3)

            def _top_up():
                while queue and len(dma_lead) < LEAD:
                    u = queue.popleft()
                    if u[2] is not None:
                        u[2]()
                    dma_lead.append(u)

            def pull_one():
                _top_up()
                if dma_lead:
                    u = dma_lead.popleft()
                    u[3]()
                    _top_up()

            def flush(pred):
                _top_up()
                while dma_lead and pred(dma_lead[0]):
                    u = dma_lead.popleft()
                    u[3]()
                    _top_up()

            def push_units(b):
                for u in make_units(b):
                    queue.append(u)
                _top_up()

            # ---------- attention ----------
            def attn_batch(b):
                QT, KTs, Vn = QTd.pop(b), KTd.pop(b), Vnd.pop(b)
                for qbp in range(NQP):
                    q0 = qbp * 2 * QB
                    sl0 = slice(q0, q0 + QB)
                    sl1 = slice(q0 + QB, q0 + 2 * QB)
                    ytile = work.tile([128, 2 * QB], f32, tag="ytile", name="ytile")
                    for h in range(HPC):
                        if h == 1:
                            flush(lambda u: u[0] < b or (u[0] == b and u[1] == "B"))
                        pctx = psC.tile([128, 2 * QB], f32, tag="pCtx", name="pctx", bufs=1)
                        # rowsum accumulators: pair-wide adds split across the
                        # vector and gpsimd engines (DVE alone can't keep up
                        # with fp8-rate adds)
                        # flat 2D accumulators: 3D-strided adds run ~3x slower
                        acc_v = work.tile([128, 4 * QB], bf16, tag="acc_v", name="acc_v")
                        acc_g = work.tile([128, 4 * QB], bf16, tag="acc_g", name="acc_g")
                        ptiles = []

                        def emit_pv(pair):
                            # PV software-pipelined one pair behind the QK/exp
                            # chain so it overlaps the next pair's exps instead
                            # of serializing inside the dependency loop
                            for half in range(2):
                                hs = slice(half * QB, (half + 1) * QB)
                                nc.tensor.matmul(
                                    pctx[:, hs], Vn[h][:, pair], ptiles[pair][:, :, hs],
                                    start=(pair == 0), stop=(pair == NKP - 1),
                                    perf_mode=DR,
                                )

                        for pair in range(NKP):
                            pexp2 = pexpp.tile([128, 2, 2 * QB], f8, tag="pexp", bufs=8)
                            ptiles.append(pexp2)
                            for sub in range(2):
                                kt = 2 * pair + sub
                                ps2 = psS.tile([128, 2 * QB], f32, tag="pS", name="ps2", bufs=2)
                                ksl = slice(kt * 128, (kt + 1) * 128)
                                nc.tensor.matmul(
                                    ps2[:, :QB], KTs[h][:, ksl], QT[h][:, sl0],
                                    start=True, stop=True,
                                )
                                nc.tensor.matmul(
                                    ps2[:, QB:], KTs[h][:, ksl], QT[h][:, sl1],
                                    start=True, stop=True,
                                )
                                nc.scalar.activation(
                                    pexp2[:, sub, :], ps2[:], AF.Exp, scale=scale
                                )
                            if pair > 0:
                                emit_pv(pair - 1)
                            pull_one()  # proj filler where PE waits on exp

                            def pm(i):
                                return ptiles[i][:].rearrange("p a q -> p (a q)")

                            # last pair's add on the faster DVE, split per-kt,
                            # so the rowsum collapse isn't stalled at the head
                            # boundary
                            with nc.allow_low_precision(reason="bf16 rowsum acc"):
                                if pair == 2:
                                    nc.vector.tensor_add(acc_v[:], pm(0), pm(2))
                                elif pair == 3:
                                    nc.gpsimd.tensor_add(acc_g[:], pm(1), pm(3))
                                elif pair == 5:
                                    nc.gpsimd.tensor_add(acc_g[:], acc_g[:], pm(5))
                                elif pair in (4, 6):
                                    nc.vector.tensor_add(acc_v[:], acc_v[:], pm(pair))
                                elif pair == 7:
                                    for sub in range(2):
                                        ss = slice(sub * 2 * QB, (sub + 1) * 2 * QB)
                                        nc.vector.tensor_add(
                                            acc_v[:, ss], acc_v[:, ss],
                                            ptiles[7][:, sub, :],
                                        )
                        emit_pv(NKP - 1)
                        pull_one()
                        # rowsum collapse + normalize, pipelined per 512-half
                        rsbr = work.tile([128, 2 * QB], f32, tag="rsbr", name="rsbr", bufs=2)
                        ctxn = None if h == 0 else work.tile(
                            [128, 2 * QB], f32, tag="ctxn", name="ctxn"
                        )
                        for half in range(2):
                            hs = slice(half * QB, (half + 1) * QB)
                            pbc = psP.tile([128, QB], f32, tag="pP", name="pbc", bufs=2)
                            srcs = [
                                acc_v[:, half * QB : (half + 1) * QB],
                                acc_v[:, 2 * QB + half * QB : 2 * QB + (half + 1) * QB],
                                acc_g[:, half * QB : (half + 1) * QB],
                                acc_g[:, 2 * QB + half * QB : 2 * QB + (half + 1) * QB],
                            ]
                            for si, src in enumerate(srcs):
                                nc.tensor.matmul(
                                    pbc[:], onemb_sb[:], src,
                                    start=(si == 0), stop=(si == len(srcs) - 1),
                                )
                            nc.vector.reciprocal_approx_fast(out=rsbr[:, hs], in_=pbc[:])
                            if h == 0:
                                nc.vector.tensor_mul(ytile[:, hs], pctx[:, hs], rsbr[:, hs])
                            else:
                                nc.vector.tensor_mul(ctxn[:, hs], pctx[:, hs], rsbr[:, hs])
                                nc.vector.tensor_add(
                                    ytile[:, hs], ytile[:, hs], ctxn[:, hs]
                                )
                                nc.scalar.dma_start(
                                    y_bounce[b][qbp][:, hs], ytile[:, hs]
                                )
                            if half == 0:
                                pull_one()
                        if h == 1:
                            nc.gpsimd.collective_compute(
                                "ReduceScatter",
                                mybir.AluOpType.add,
                                replica_groups=[list(range(NCORES))],
                                ins=[y_bounce[b][qbp][:].opt()],
                                outs=[y_shard[b][qbp][:].opt()],
                            )

            # ---------- schedule ----------
            push_units(0)
            flush(lambda u: u[0] == 0 and u[1] == "A")
            for b in range(B):
                if b + 1 < B:
                    push_units(b + 1)
                attn_batch(b)
                flush(lambda u: u[0] <= b)
            flush(lambda u: True)
            # out dmas at the end: each waits its RS; nothing queues behind
            for b in range(B):
                for qbp in range(NQP):
                    nc.sync.dma_start(out_y[b, qbp], y_shard[b][qbp][:])

    nc.compile()
    return nc


def kernel(**inputs):
    query = np.asarray(inputs["query"], np.float32)
    key = np.asarray(inputs["key"], np.float32)
    Wq, bq = np.asarray(inputs["Wq"], np.float32), np.asarray(inputs["bq"], np.float32)
    Wk, bk = np.asarray(inputs["Wk"], np.float32), np.asarray(inputs["bk"], np.float32)
    Wv, bv = np.asarray(inputs["Wv"], np.float32), np.asarray(inputs["bv"], np.float32)
    Wp, bp = np.asarray(inputs["Wp"], np.float32), np.asarray(inputs["bp"], np.float32)
    Wo, bo = np.asarray(inputs["Wo"], np.float32), np.asarray(inputs["bo"], np.float32)

    f8np = ml_dtypes.float8_e4m3
    qT_8 = np.ascontiguousarray(query.transpose(0, 2, 1)).astype(f8np)
    kT_8 = np.ascontiguousarray(key.transpose(0, 2, 1)).astype(f8np)

    if "nc" not in _cache:
        _cache["nc"] = build()
    nc = _cache["nc"]

    def prepack_qk(w):  # 32x-scaled [HPC, IN, D] -> [128, HPC, NCH, D] fp8
        return np.ascontiguousarray(
            (32.0 * w).reshape(HPC, NCH, 128, D).transpose(2, 0, 1, 3)
        ).astype(f8np)

    def prepack_v(w):  # 64x-scaled [HPC, IN, D] -> [128, NCH, HPC, D] fp8
        return np.ascontiguousarray(
            (64.0 * w).reshape(HPC, NCH, 128, D).transpose(2, 1, 0, 3)
        ).astype(f8np)

    Wo_h = Wo.reshape(H, D, D)
    bias_total = (
        np.einsum("hd,hde,hef->f", bv.astype(np.float64), Wp.astype(np.float64), Wo_h.astype(np.float64))
        + np.einsum("hd,hdf->f", bp.astype(np.float64), Wo_h.astype(np.float64))
        + bo.astype(np.float64)
    ).astype(np.float32)

    in_maps = []
    for i in range(NCORES):
        hs = slice(i * HPC, (i + 1) * HPC)
        wvpp = np.einsum(
            "hid,hde,hef->hif",
            Wv[hs].astype(np.float64),
            Wp[hs].astype(np.float64),
            Wo_h[hs].astype(np.float64),
        ).astype(np.float32)
        in_maps.append(
            {
                "qT": qT_8,
                "kT": kT_8,
                "wq": prepack_qk(Wq[hs]),
                "wk": prepack_qk(Wk[hs]),
                "wv": prepack_v(wvpp),
                "bqT": np.ascontiguousarray(32.0 * bq[hs].T),
                "bkT": np.ascontiguousarray(32.0 * bk[hs].T),
                "onemb": np.ones((D, D), ml_dtypes.bfloat16),
            }
        )

    res = run_bass_kernel_spmd(nc, in_maps, core_ids=list(range(NCORES)))
    _cache["last_result"] = res
    # shards: per core [B, NQP, ESH, 2QB] -> full [B, S, D]; y is 64x-scaled
    parts = np.stack([res.results[i]["out_y"] for i in range(NCORES)], axis=2)
    yfull = parts.reshape(B, NQP, D, 2 * QB).transpose(0, 1, 3, 2).reshape(B, S, D)
    return np.ascontiguousarray(yfull / 64.0 + bias_total[None, None, :])
